# revision 1
# baseline (speedup 1.0000x reference)
"""Multi-head attention (16 heads, d_model=1024, B=2, S=2048) on 8 trn2 cores.

Sharding: head-parallel. Core c handles batch b = c // 4 and the 4 heads
g = c % 4 (head indices 4g..4g+3). Each core computes:
  - qh/kh/vh projections for its heads (contraction over full d_model)
  - scores + softmax + attn_weights output for its heads
  - attn_out @ W_o[:, its head columns].T  -> a [S, d_model] partial output
Host sums the 4 partial outputs per batch (row-sharded W_o reduction) and
concatenates attn_weights.

Device kernel (per core, same SPMD program):
  inputs (host pre-transposed, bf16):
    qT, kT, vT  [1024, S]    (x^T so the d_model contraction sits on partitions)
    wqT wkT wvT [1024, 256]  (W[4 heads' rows, :]^T)
    woT         [256, 1024]  (W_o[:, 4 heads' cols]^T)
  outputs (f32): attn_w [4, S, S], out_partial [S, 1024]

  Phase A: qhT/khT [2 head-pairs][128p=2x64d, S] and vh [S, 256] via PE.
           1/sqrt(d_k) folded into qhT copy.
  Pass 1 (per head, per 128-row q-tile): scores = qhT^T @ khT (K=64 on
           partitions 0-63/64-127 so even/odd heads pack the PE array),
           exp on ACT with accum_out row-sums, reciprocal, normalize on DVE,
           DMA 1MB attn_w tiles.
  Recip bounce: [128, qt] recip tiles -> DRAM -> [1, S] q-major rows (needed
           as free-dim vectors for the PV normalization outer-product).
  Pass 2 (per head, per 512-col q-chunk): scores^T tiles [128k, 512q] -> exp
           -> PV accumulate attn_outT [64d, 512q] in PSUM; normalize by
           ones x recip outer-product (PE) + DVE multiply.
  Outproj: out[q, :] = sum_h attn_outT_h^T @ woT_h, K=64 accumulation.
"""

import numpy as np
import ml_dtypes

import concourse.bacc as bacc
import concourse.mybir as mybir
from concourse.tile import TileContext
from concourse.bass_utils import run_bass_kernel_spmd

D_MODEL = 1024
H_TOTAL = 16
D_K = 64
NH = 4               # heads per core
HD = NH * D_K        # 256
N_CORES = 8

BF16 = mybir.dt.bfloat16
F32 = mybir.dt.float32
AF = mybir.ActivationFunctionType
AX = mybir.AxisListType


def build_program(S=2048):
    SC = min(512, S)         # free-dim chunk (one PSUM bank of f32)
    NSC = S // SC
    NQT = S // 128
    NKT = S // 128
    NDT = D_MODEL // 128
    NOC = D_MODEL // SC

    nc = bacc.Bacc(None, target_bir_lowering=False, debug=False)
    qT = nc.declare_dram_parameter("qT", [D_MODEL, S], BF16, isOutput=False)
    kT = nc.declare_dram_parameter("kT", [D_MODEL, S], BF16, isOutput=False)
    vT = nc.declare_dram_parameter("vT", [D_MODEL, S], BF16, isOutput=False)
    wqT = nc.declare_dram_parameter("wqT", [D_MODEL, HD], BF16, isOutput=False)
    wkT = nc.declare_dram_parameter("wkT", [D_MODEL, HD], BF16, isOutput=False)
    wvT = nc.declare_dram_parameter("wvT", [D_MODEL, HD], BF16, isOutput=False)
    woT = nc.declare_dram_parameter("woT", [HD, D_MODEL], BF16, isOutput=False)
    attn_w = nc.declare_dram_parameter("attn_w", [NH, S, S], F32, isOutput=True)
    out_partial = nc.declare_dram_parameter(
        "out_partial", [S, D_MODEL], F32, isOutput=True
    )

    with TileContext(nc) as tc:
        with (
            tc.tile_pool(name="w", bufs=1) as wpool,
            tc.tile_pool(name="acts", bufs=1) as apool,
            tc.tile_pool(name="psmm", bufs=4, space="PSUM") as ps_mm,
            tc.tile_pool(name="pspv", bufs=1, space="PSUM") as ps_pv_pool,
            tc.tile_pool(name="psbc", bufs=1, space="PSUM") as ps_bc_pool,
            tc.tile_pool(name="pso", bufs=2, space="PSUM") as ps_o_pool,
            tc.tile_pool(name="dram", bufs=1, space="DRAM") as dpool,
        ):
            # persistent SBUF
            wq_sb = wpool.tile([128, NDT, HD], BF16)
            wk_sb = wpool.tile([128, NDT, HD], BF16)
            wv_sb = wpool.tile([128, NDT, HD], BF16)
            wo_sb = wpool.tile([64, NH, D_MODEL], BF16)
            ones_sb = wpool.tile([1, 64], F32)
            qhT = apool.tile([128, 2, S], BF16)
            khT = apool.tile([128, 2, S], BF16)
            vh = apool.tile([128, NKT, HD], BF16)
            aoT = apool.tile([64, NH, S], BF16)
            recip_sb = apool.tile([128, NH, NQT], F32)
            rb = dpool.tile([NH, 1, S], F32)

            nc.vector.memset(ones_sb[:], 1.0)
            nc.sync.dma_start(
                out=wq_sb[:], in_=wqT[:, :].rearrange("(t p) m -> p t m", p=128)
            )
            nc.sync.dma_start(
                out=wk_sb[:], in_=wkT[:, :].rearrange("(t p) m -> p t m", p=128)
            )
            nc.sync.dma_start(
                out=wv_sb[:], in_=wvT[:, :].rearrange("(t p) m -> p t m", p=128)
            )
            nc.sync.dma_start(
                out=wo_sb[:], in_=woT[:, :].rearrange("(h d) o -> d h o", d=64)
            )

            # ---- Phase A: projections ----
            with tc.tile_pool(name="xin", bufs=2) as xpool:
                for src, wsb, dstT, scale in (
                    (qT, wq_sb, qhT, 1.0 / np.sqrt(D_K)),
                    (kT, wk_sb, khT, 1.0),
                ):
                    xin = xpool.tile([128, NDT, S], BF16, tag="xin")
                    nc.sync.dma_start(
                        out=xin[:], in_=src[:, :].rearrange("(t p) s -> p t s", p=128)
                    )
                    for m in range(2):
                        for sc_i in range(NSC):
                            ps = ps_mm.tile([128, SC], F32, tag="mm")
                            for t in range(NDT):
                                nc.tensor.matmul(
                                    ps[:],
                                    lhsT=wsb[:, t, m * 128 : (m + 1) * 128],
                                    rhs=xin[:, t, sc_i * SC : (sc_i + 1) * SC],
                                    start=(t == 0),
                                    stop=(t == NDT - 1),
                                )
                            if scale != 1.0:
                                nc.scalar.mul(
                                    dstT[:, m, sc_i * SC : (sc_i + 1) * SC], ps[:], scale
                                )
                            else:
                                nc.vector.tensor_copy(
                                    dstT[:, m, sc_i * SC : (sc_i + 1) * SC], ps[:]
                                )
                xin = xpool.tile([128, NDT, S], BF16, tag="xin")
                nc.sync.dma_start(
                    out=xin[:], in_=vT[:, :].rearrange("(t p) s -> p t s", p=128)
                )
                for st in range(NKT):
                    ps = ps_mm.tile([128, HD], F32, tag="mm")
                    for t in range(NDT):
                        nc.tensor.matmul(
                            ps[:],
                            lhsT=xin[:, t, st * 128 : (st + 1) * 128],
                            rhs=wv_sb[:, t, :],
                            start=(t == 0),
                            stop=(t == NDT - 1),
                        )
                    nc.vector.tensor_copy(vh[:, st, :], ps[:])

            # ---- Attention ----
            with (
                tc.tile_pool(name="p1", bufs=2) as p1pool,
                tc.tile_pool(name="p2", bufs=4) as p2pool,
                tc.tile_pool(name="rr", bufs=2) as rrpool,
                tc.tile_pool(name="osb", bufs=2) as opool,
            ):
                for hp in range(2):
                    heads = (2 * hp, 2 * hp + 1)
                    # pass 1: attn_w rows + softmax denominators
                    for qt in range(NQT):
                        for h in heads:
                            bp = 64 * (h % 2)
                            pair = h // 2
                            expP = p1pool.tile([128, S], BF16, tag="expP")
                            rs4 = p1pool.tile([128, NSC + 1], F32, tag="rs4")
                            for kc in range(NSC):
                                ps = ps_mm.tile([128, SC], F32, tag="mm")
                                nc.tensor.matmul(
                                    ps[:],
                                    lhsT=qhT[bp : bp + 64, pair, qt * 128 : (qt + 1) * 128],
                                    rhs=khT[bp : bp + 64, pair, kc * SC : (kc + 1) * SC],
                                    start=True,
                                    stop=True,
                                )
                                nc.scalar.activation(
                                    expP[:, kc * SC : (kc + 1) * SC],
                                    ps[:],
                                    AF.Exp,
                                    accum_out=rs4[:, kc : kc + 1],
                                )
                            nc.vector.reduce_sum(
                                rs4[:, NSC : NSC + 1], rs4[:, 0:NSC], axis=AX.X
                            )
                            nc.vector.reciprocal(
                                recip_sb[:, h, qt : qt + 1], rs4[:, NSC : NSC + 1]
                            )
                            aw = p1pool.tile([128, S], F32, tag="aw")
                            nc.vector.tensor_scalar_mul(
                                aw[:], expP[:], recip_sb[:, h, qt : qt + 1]
                            )
                            nc.sync.dma_start(
                                out=attn_w[h, qt * 128 : (qt + 1) * 128, :], in_=aw[:]
                            )
                    # bounce recips to q-major [1, S] rows
                    rrs = {}
                    for h in heads:
                        nc.sync.dma_start(
                            out=rb[h, 0].rearrange("(a p) -> p a", p=128),
                            in_=recip_sb[:, h, :],
                        )
                        rr = rrpool.tile([1, S], F32, tag="rr")
                        nc.sync.dma_start(out=rr[:], in_=rb[h])
                        rrs[h] = rr
                    # pass 2: scores^T -> exp -> PV, normalized into aoT
                    for qc in range(NSC):
                        for h in heads:
                            bp = 64 * (h % 2)
                            pair = h // 2
                            pv = ps_pv_pool.tile([64, SC], F32, tag="pv")
                            for kt in range(NKT):
                                ps2 = ps_mm.tile([128, SC], F32, tag="mm")
                                nc.tensor.matmul(
                                    ps2[:],
                                    lhsT=khT[bp : bp + 64, pair, kt * 128 : (kt + 1) * 128],
                                    rhs=qhT[bp : bp + 64, pair, qc * SC : (qc + 1) * SC],
                                    start=True,
                                    stop=True,
                                )
                                ept = p2pool.tile([128, SC], BF16, tag="ept")
                                nc.scalar.activation(ept[:], ps2[:], AF.Exp)
                                nc.tensor.matmul(
                                    pv[:],
                                    lhsT=vh[:, kt, h * 64 : (h + 1) * 64],
                                    rhs=ept[:],
                                    start=(kt == 0),
                                    stop=(kt == NKT - 1),
                                )
                            bc = ps_bc_pool.tile([64, SC], F32, tag="bc")
                            nc.tensor.matmul(
                                bc[:],
                                lhsT=ones_sb[:, :],
                                rhs=rrs[h][:, qc * SC : (qc + 1) * SC],
                                start=True,
                                stop=True,
                            )
                            bc_sb = p2pool.tile([64, SC], F32, tag="bcsb")
                            nc.vector.tensor_copy(bc_sb[:], bc[:])
                            nc.vector.tensor_mul(
                                aoT[:, h, qc * SC : (qc + 1) * SC], pv[:], bc_sb[:]
                            )
                # outproj
                for qt in range(NQT):
                    osb = opool.tile([128, D_MODEL], F32, tag="osb")
                    for nh in range(NOC):
                        pso = ps_o_pool.tile([128, SC], F32, tag="o")
                        for h in range(NH):
                            nc.tensor.matmul(
                                pso[:],
                                lhsT=aoT[:, h, qt * 128 : (qt + 1) * 128],
                                rhs=wo_sb[:, h, nh * SC : (nh + 1) * SC],
                                start=(h == 0),
                                stop=(h == NH - 1),
                            )
                        nc.vector.tensor_copy(osb[:, nh * SC : (nh + 1) * SC], pso[:])
                    nc.sync.dma_start(
                        out=out_partial[qt * 128 : (qt + 1) * 128, :], in_=osb[:]
                    )
    return nc


_prog_cache = {}


def _get_program(S):
    if S not in _prog_cache:
        nc = build_program(S)
        nc.compile()
        _prog_cache[S] = nc
    return _prog_cache[S]


def _prep_in_maps(q, k, v, W_q, W_k, W_v, W_o):
    bf = ml_dtypes.bfloat16
    xs = {}
    for b in range(2):
        xs[b] = {
            "qT": np.ascontiguousarray(q[b].T).astype(bf),
            "kT": np.ascontiguousarray(k[b].T).astype(bf),
            "vT": np.ascontiguousarray(v[b].T).astype(bf),
        }
    in_maps = []
    for c in range(N_CORES):
        b, g = divmod(c, 4)
        sl = slice(g * HD, (g + 1) * HD)
        in_maps.append(
            {
                **xs[b],
                "wqT": np.ascontiguousarray(W_q[sl].T).astype(bf),
                "wkT": np.ascontiguousarray(W_k[sl].T).astype(bf),
                "wvT": np.ascontiguousarray(W_v[sl].T).astype(bf),
                "woT": np.ascontiguousarray(W_o[:, sl].T).astype(bf),
            }
        )
    return in_maps


def kernel(q, k, v, W_q, W_k, W_v, W_o):
    q = np.asarray(q, np.float32)
    k = np.asarray(k, np.float32)
    v = np.asarray(v, np.float32)
    W_q = np.asarray(W_q, np.float32)
    W_k = np.asarray(W_k, np.float32)
    W_v = np.asarray(W_v, np.float32)
    W_o = np.asarray(W_o, np.float32)
    B, S, _ = q.shape

    nc = _get_program(S)
    in_maps = _prep_in_maps(q, k, v, W_q, W_k, W_v, W_o)
    res = run_bass_kernel_spmd(nc, in_maps, list(range(N_CORES))).results

    attn = np.empty((B, H_TOTAL, S, S), np.float32)
    out = np.zeros((B, S, D_MODEL), np.float32)
    for c in range(N_CORES):
        b, g = divmod(c, 4)
        attn[b, g * NH : (g + 1) * NH] = res[c]["attn_w"]
        out[b] += res[c]["out_partial"]
    return out, attn


# revision 2
# speedup vs baseline: 1.0936x; 1.0936x over previous
"""Multi-head attention (16 heads, d_model=1024, B=2, S=2048) on 8 trn2 cores.

Sharding: head-parallel. Core c handles batch b = c // 4 and the 4 heads
g = c % 4 (head indices 4g..4g+3). Each core computes:
  - qh/kh/vh projections for its heads (contraction over full d_model)
  - scores + softmax + attn_weights output for its heads
  - attn_out @ W_o[:, its head columns].T  -> a [S, d_model] partial output
Host sums the 4 partial outputs per batch (row-sharded W_o reduction) and
concatenates attn_weights.

Device kernel (per core, same SPMD program):
  inputs (host pre-transposed, bf16):
    qT, kT, vT  [1024, S]    (x^T so the d_model contraction sits on partitions)
    wqT wkT wvT [1024, 256]  (W[4 heads' rows, :]^T)
    woT         [256, 1024]  (W_o[:, 4 heads' cols]^T)
  outputs (f32): attn_w [4, S, S], out_partial [S, 1024]

  Phase A: qhT/khT [2 head-pairs][128p=2x64d, S] and vh [S, 256] via PE.
           1/sqrt(d_k) folded into qhT copy.
  Pass 1 (per head, per 128-row q-tile): scores into [128, 1024] 2-bank PSUM
           tiles (K=64 on partitions 0-63/64-127 so even/odd heads can pack
           the PE array), wide exp on ACT with accum_out row-sums,
           reciprocal, normalize on DVE, DMA 1MB attn_w tiles.
  Recip bounce: [128, qt] recip tiles -> DRAM -> [1, S] q-major rows.
  Pass 2 (per head, per 1024-col q-chunk): scores^T [128k, 1024q] -> wide exp
           -> PV accumulate attn_outT [64d, 1024q] in PSUM; normalize with a
           GPSIMD partition_broadcast of the recip row + DVE multiply.
  Outproj: out[q, :] = sum_h attn_outT_h^T @ woT_h, K=64 accumulation
           (PSUM slots shared with the PV tag).
"""

import numpy as np
import ml_dtypes

import concourse.bacc as bacc
import concourse.mybir as mybir
from concourse.tile import TileContext
from concourse.bass_utils import run_bass_kernel_spmd

D_MODEL = 1024
H_TOTAL = 16
D_K = 64
NH = 4               # heads per core
HD = NH * D_K        # 256
N_CORES = 8

BF16 = mybir.dt.bfloat16
F32 = mybir.dt.float32
AF = mybir.ActivationFunctionType
AX = mybir.AxisListType


def build_program(S=2048):
    MM = min(512, S)         # max matmul free dim into one PSUM bank (f32)
    CH = min(1024, S)        # exp / PV chunk (2 PSUM banks)
    NCH = S // CH            # chunks per row
    NMM = CH // MM           # matmuls per chunk
    NQT = S // 128
    NKT = S // 128
    NDT = D_MODEL // 128
    NOC = D_MODEL // MM

    nc = bacc.Bacc(None, target_bir_lowering=False, debug=False)
    qT = nc.declare_dram_parameter("qT", [D_MODEL, S], BF16, isOutput=False)
    kT = nc.declare_dram_parameter("kT", [D_MODEL, S], BF16, isOutput=False)
    vT = nc.declare_dram_parameter("vT", [D_MODEL, S], BF16, isOutput=False)
    wqT = nc.declare_dram_parameter("wqT", [D_MODEL, HD], BF16, isOutput=False)
    wkT = nc.declare_dram_parameter("wkT", [D_MODEL, HD], BF16, isOutput=False)
    wvT = nc.declare_dram_parameter("wvT", [D_MODEL, HD], BF16, isOutput=False)
    woT = nc.declare_dram_parameter("woT", [HD, D_MODEL], BF16, isOutput=False)
    attn_w = nc.declare_dram_parameter("attn_w", [NH, S, S], F32, isOutput=True)
    out_partial = nc.declare_dram_parameter(
        "out_partial", [S, D_MODEL], F32, isOutput=True
    )

    with TileContext(nc) as tc:
        with (
            tc.tile_pool(name="w", bufs=1) as wpool,
            tc.tile_pool(name="acts", bufs=1) as apool,
            tc.tile_pool(name="dram", bufs=1, space="DRAM") as dpool,
        ):
            # persistent SBUF
            wq_sb = wpool.tile([128, NDT, HD], BF16)
            wk_sb = wpool.tile([128, NDT, HD], BF16)
            wv_sb = wpool.tile([128, NDT, HD], BF16)
            wo_sb = wpool.tile([64, NH, D_MODEL], BF16)
            qhT = apool.tile([128, 2, S], BF16)
            khT = apool.tile([128, 2, S], BF16)
            vh = apool.tile([128, NKT, HD], BF16)
            aoT = apool.tile([64, NH, S], BF16)
            recip_sb = apool.tile([128, NH, NQT], F32)
            rb = dpool.tile([NH, 1, S], F32)

            nc.sync.dma_start(
                out=wq_sb[:], in_=wqT[:, :].rearrange("(t p) m -> p t m", p=128)
            )
            nc.sync.dma_start(
                out=wk_sb[:], in_=wkT[:, :].rearrange("(t p) m -> p t m", p=128)
            )
            nc.sync.dma_start(
                out=wv_sb[:], in_=wvT[:, :].rearrange("(t p) m -> p t m", p=128)
            )
            nc.sync.dma_start(
                out=wo_sb[:], in_=woT[:, :].rearrange("(h d) o -> d h o", d=64)
            )

            # ---- Phase A: projections ----
            with (
                tc.tile_pool(name="xin", bufs=2) as xpool,
                tc.tile_pool(name="pspr", bufs=3, space="PSUM") as ps_pr,
            ):
                for src, wsb, dstT, scale in (
                    (qT, wq_sb, qhT, 1.0 / np.sqrt(D_K)),
                    (kT, wk_sb, khT, 1.0),
                ):
                    xin = xpool.tile([128, NDT, S], BF16, tag="xin")
                    nc.sync.dma_start(
                        out=xin[:], in_=src[:, :].rearrange("(t p) s -> p t s", p=128)
                    )
                    for m in range(2):
                        for sc_i in range(S // MM):
                            ps = ps_pr.tile([128, MM], F32, tag="pr")
                            for t in range(NDT):
                                nc.tensor.matmul(
                                    ps[:],
                                    lhsT=wsb[:, t, m * 128 : (m + 1) * 128],
                                    rhs=xin[:, t, sc_i * MM : (sc_i + 1) * MM],
                                    start=(t == 0),
                                    stop=(t == NDT - 1),
                                )
                            if scale != 1.0:
                                nc.vector.tensor_scalar_mul(
                                    dstT[:, m, sc_i * MM : (sc_i + 1) * MM], ps[:], scale
                                )
                            else:
                                nc.vector.tensor_copy(
                                    dstT[:, m, sc_i * MM : (sc_i + 1) * MM], ps[:]
                                )
                xin = xpool.tile([128, NDT, S], BF16, tag="xin")
                nc.sync.dma_start(
                    out=xin[:], in_=vT[:, :].rearrange("(t p) s -> p t s", p=128)
                )
                for st in range(NKT):
                    ps = ps_pr.tile([128, HD], F32, tag="pr")
                    for t in range(NDT):
                        nc.tensor.matmul(
                            ps[:],
                            lhsT=xin[:, t, st * 128 : (st + 1) * 128],
                            rhs=wv_sb[:, t, :],
                            start=(t == 0),
                            stop=(t == NDT - 1),
                        )
                    nc.vector.tensor_copy(vh[:, st, :], ps[:])

            # ---- Attention ----
            with (
                tc.tile_pool(name="p1", bufs=2) as p1pool,
                tc.tile_pool(name="p2", bufs=3) as p2pool,
                tc.tile_pool(name="rr", bufs=2) as rrpool,
                tc.tile_pool(name="osb", bufs=2) as opool,
                tc.tile_pool(name="psbig", bufs=2, space="PSUM") as ps_big,
                tc.tile_pool(name="pspvo", bufs=2, space="PSUM") as ps_pvo,
            ):
                for hp in range(2):
                    heads = (2 * hp, 2 * hp + 1)
                    # pass 1: attn_w rows + softmax denominators
                    for qt in range(NQT):
                        for h in heads:
                            bp = 64 * (h % 2)
                            pair = h // 2
                            expP = p1pool.tile([128, S], BF16, tag="expP")
                            rsc = p1pool.tile([128, NCH + 1], F32, tag="rsc")
                            for c in range(NCH):
                                ps = ps_big.tile([128, CH], F32, tag="big")
                                for i in range(NMM):
                                    nc.tensor.matmul(
                                        ps[:, i * MM : (i + 1) * MM],
                                        lhsT=qhT[
                                            bp : bp + 64, pair, qt * 128 : (qt + 1) * 128
                                        ],
                                        rhs=khT[
                                            bp : bp + 64,
                                            pair,
                                            c * CH + i * MM : c * CH + (i + 1) * MM,
                                        ],
                                        start=True,
                                        stop=True,
                                    )
                                nc.scalar.activation(
                                    expP[:, c * CH : (c + 1) * CH],
                                    ps[:],
                                    AF.Exp,
                                    accum_out=rsc[:, c : c + 1],
                                )
                            nc.vector.reduce_sum(
                                rsc[:, NCH : NCH + 1], rsc[:, 0:NCH], axis=AX.X
                            )
                            nc.vector.reciprocal(
                                recip_sb[:, h, qt : qt + 1], rsc[:, NCH : NCH + 1]
                            )
                            aw = p1pool.tile([128, S], F32, tag="aw")
                            nc.vector.tensor_scalar_mul(
                                aw[:], expP[:], recip_sb[:, h, qt : qt + 1]
                            )
                            nc.sync.dma_start(
                                out=attn_w[h, qt * 128 : (qt + 1) * 128, :], in_=aw[:]
                            )
                    # bounce recips to q-major [1, S] rows
                    rrs = {}
                    for h in heads:
                        nc.sync.dma_start(
                            out=rb[h, 0].rearrange("(a p) -> p a", p=128),
                            in_=recip_sb[:, h, :],
                        )
                        rr = rrpool.tile([1, S], F32, tag="rr")
                        nc.sync.dma_start(out=rr[:], in_=rb[h])
                        rrs[h] = rr
                    # pass 2: scores^T -> exp -> PV, normalized into aoT
                    for qc in range(NCH):
                        for h in heads:
                            bp = 64 * (h % 2)
                            pair = h // 2
                            pv = ps_pvo.tile([64, CH], F32, tag="pvo")
                            for kt in range(NKT):
                                ps2 = ps_big.tile([128, CH], F32, tag="big")
                                for i in range(NMM):
                                    nc.tensor.matmul(
                                        ps2[:, i * MM : (i + 1) * MM],
                                        lhsT=khT[
                                            bp : bp + 64, pair, kt * 128 : (kt + 1) * 128
                                        ],
                                        rhs=qhT[
                                            bp : bp + 64,
                                            pair,
                                            qc * CH + i * MM : qc * CH + (i + 1) * MM,
                                        ],
                                        start=True,
                                        stop=True,
                                    )
                                ept = p2pool.tile([128, CH], BF16, tag="ept")
                                nc.scalar.activation(ept[:], ps2[:], AF.Exp)
                                for i in range(NMM):
                                    nc.tensor.matmul(
                                        pv[:, i * MM : (i + 1) * MM],
                                        lhsT=vh[:, kt, h * 64 : (h + 1) * 64],
                                        rhs=ept[:, i * MM : (i + 1) * MM],
                                        start=(kt == 0),
                                        stop=(kt == NKT - 1),
                                    )
                            bc_sb = p2pool.tile([64, CH], F32, tag="bcsb")
                            nc.gpsimd.partition_broadcast(
                                bc_sb[:], rrs[h][:, qc * CH : (qc + 1) * CH], channels=64
                            )
                            nc.vector.tensor_mul(
                                aoT[:, h, qc * CH : (qc + 1) * CH], pv[:], bc_sb[:]
                            )
                # outproj
                for qt in range(NQT):
                    osb = opool.tile([128, D_MODEL], F32, tag="osb")
                    for no in range(NOC):
                        pso = ps_pvo.tile([128, MM], F32, tag="pvo")
                        for h in range(NH):
                            nc.tensor.matmul(
                                pso[:],
                                lhsT=aoT[:, h, qt * 128 : (qt + 1) * 128],
                                rhs=wo_sb[:, h, no * MM : (no + 1) * MM],
                                start=(h == 0),
                                stop=(h == NH - 1),
                            )
                        nc.vector.tensor_copy(osb[:, no * MM : (no + 1) * MM], pso[:])
                    nc.sync.dma_start(
                        out=out_partial[qt * 128 : (qt + 1) * 128, :], in_=osb[:]
                    )
    return nc


_prog_cache = {}


def _get_program(S):
    if S not in _prog_cache:
        nc = build_program(S)
        nc.compile()
        _prog_cache[S] = nc
    return _prog_cache[S]


def _prep_in_maps(q, k, v, W_q, W_k, W_v, W_o):
    bf = ml_dtypes.bfloat16
    xs = {}
    for b in range(2):
        xs[b] = {
            "qT": np.ascontiguousarray(q[b].T).astype(bf),
            "kT": np.ascontiguousarray(k[b].T).astype(bf),
            "vT": np.ascontiguousarray(v[b].T).astype(bf),
        }
    in_maps = []
    for c in range(N_CORES):
        b, g = divmod(c, 4)
        sl = slice(g * HD, (g + 1) * HD)
        in_maps.append(
            {
                **xs[b],
                "wqT": np.ascontiguousarray(W_q[sl].T).astype(bf),
                "wkT": np.ascontiguousarray(W_k[sl].T).astype(bf),
                "wvT": np.ascontiguousarray(W_v[sl].T).astype(bf),
                "woT": np.ascontiguousarray(W_o[:, sl].T).astype(bf),
            }
        )
    return in_maps


def kernel(q, k, v, W_q, W_k, W_v, W_o):
    q = np.asarray(q, np.float32)
    k = np.asarray(k, np.float32)
    v = np.asarray(v, np.float32)
    W_q = np.asarray(W_q, np.float32)
    W_k = np.asarray(W_k, np.float32)
    W_v = np.asarray(W_v, np.float32)
    W_o = np.asarray(W_o, np.float32)
    B, S, _ = q.shape

    nc = _get_program(S)
    in_maps = _prep_in_maps(q, k, v, W_q, W_k, W_v, W_o)
    res = run_bass_kernel_spmd(nc, in_maps, list(range(N_CORES))).results

    attn = np.empty((B, H_TOTAL, S, S), np.float32)
    out = np.zeros((B, S, D_MODEL), np.float32)
    for c in range(N_CORES):
        b, g = divmod(c, 4)
        attn[b, g * NH : (g + 1) * NH] = res[c]["attn_w"]
        out[b] += res[c]["out_partial"]
    return out, attn
